# revision 1
# baseline (speedup 1.0000x reference)
"""Causal local self-attention (RoPE, window=512) on 8 Trainium2 NeuronCores.

Sharding: (batch, query-quarter) across 8 cores -> core c handles
b = c // 4, queries [512*(c%4), 512*(c%4)+512).  Each core computes
q/k/v projections for its key window [qe-1024, qe) (zero-padded at the
left edge), RoPE, banded softmax, attention, and its output-row slice
of the final projection.  No collectives needed.

Layouts (all chosen so no on-device transposes are ever needed):
  - Q^T, K^T [head_dim-on-partitions, time]   (projection: lhsT=W^T tile)
  - V natural [time-on-partitions, head_dim]  (projection: lhsT=x^T tile)
  - scores^T [key-on-partitions, query]       (AV: lhsT=[V|1], free denominator)
  - out natural [time, channels]              (projection: lhsT=Y^T tile)

RoPE: rotate_half is a partition-pair swap in the transposed layout,
done with stream_shuffle on the projection PSUM; the sign lives in the
host-built sin table.  V is projected first, then K/Q per o-tile with
the attention for that head-pair emitted immediately after, so the
ACT/DVE-heavy softmax work overlaps the remaining projection matmuls.
"""

import sys
sys.path.insert(0, '/opt/trn_rl_repo')

from contextlib import ExitStack

import numpy as np

from concourse import bass, bacc, mybir, tile
from concourse.bass_utils import run_bass_kernel_spmd

F32 = mybir.dt.float32

B, T, C, H, HD = 2, 2048, 1024, 16, 64
LOCAL_WINDOW = 512
ROPE_BASE = 10000.0
NCORES = 8
QL = 512     # queries per core
KL = 1024    # key window per core
QB = 256     # query block (free dim of transposed scores)
NSUB = 6     # key subtiles (of 128) per query block
SCALE = 1.0 / 8.0  # 1/sqrt(HD), folded into Wq on the host

# matmul dtype: float32 (safe) or float32r (fast, ~tf32 precision)
DT_MM = mybir.dt.float32r

SWAP_MASK = [i ^ 1 for i in range(32)]


def build_nc(reps=1):
    nc = bacc.Bacc("TRN2", target_bir_lowering=False, debug=False,
                   num_devices=NCORES)

    xT_d = nc.dram_tensor("xT", [C, KL], DT_MM, kind="ExternalInput").ap()
    wT_d = nc.dram_tensor("wT", [C, 3 * C], DT_MM, kind="ExternalInput").ap()
    wpT_d = nc.dram_tensor("wpT", [C, C], DT_MM, kind="ExternalInput").ap()
    biasb_d = nc.dram_tensor("biasb", [128, C], F32, kind="ExternalInput").ap()
    cosq_d = nc.dram_tensor("cosq", [128, QL], F32, kind="ExternalInput").ap()
    sinq_d = nc.dram_tensor("sinq", [128, QL], F32, kind="ExternalInput").ap()
    cosk_d = nc.dram_tensor("cosk", [128, KL], F32, kind="ExternalInput").ap()
    sink_d = nc.dram_tensor("sink", [128, KL], F32, kind="ExternalInput").ap()
    mask_d = nc.dram_tensor("mask", [128, 12 * QB], F32,
                            kind="ExternalInput").ap()
    vone_d = nc.dram_tensor("vone", [128, 8 * H], DT_MM,
                            kind="ExternalInput").ap()
    out_d = nc.dram_tensor("out", [QL, C], F32, kind="ExternalOutput").ap()

    with tile.TileContext(nc) as tc, ExitStack() as top:
        const = top.enter_context(tc.tile_pool(name="const", bufs=1))
        persist = top.enter_context(tc.tile_pool(name="persist", bufs=1))

        cosq_sb = const.tile([128, QL], F32, tag="cosq", name="cosq")
        sinq_sb = const.tile([128, QL], F32, tag="sinq", name="sinq")
        cosk_sb = const.tile([128, KL], F32, tag="cosk", name="cosk")
        sink_sb = const.tile([128, KL], F32, tag="sink", name="sink")
        mask_sb = const.tile([128, 12 * QB], F32, tag="mask", name="mask")
        bias_sb = const.tile([128, C], F32, tag="bias", name="bias")
        nc.sync.dma_start(out=cosq_sb, in_=cosq_d)
        nc.sync.dma_start(out=sinq_sb, in_=sinq_d)
        nc.sync.dma_start(out=cosk_sb, in_=cosk_d)
        nc.sync.dma_start(out=sink_sb, in_=sink_d)
        nc.sync.dma_start(out=mask_sb, in_=mask_d)
        nc.sync.dma_start(out=bias_sb, in_=biasb_d)

        qT = [persist.tile([128, QL], DT_MM, tag=f"qT{i}", name=f"qT{i}")
              for i in range(8)]
        yT = qT  # reuse: yT[ot] slices are written only after the
        #  corresponding qT[ot] slices' last reader (disjoint per qbi)
        kT = [persist.tile([128, KL], DT_MM, tag=f"kT{i}", name=f"kT{i}")
              for i in range(8)]
        v1 = [persist.tile([128, H * (HD + 1)], DT_MM, tag=f"v1{i}",
                           name=f"v1{i}")
              for i in range(8)]

        def rope_evict(dest_slice, psm, rpool, cosT, sinT):
            """dest = psm*cos + swap_pairs(psm)*sin_signed.

            DVE: shuffle (PSUM->SBUF) + cos-mult (PSUM read);
            GpSimd (otherwise idle): sin-mult + add (SBUF only)."""
            r = rpool.tile([128, 512], F32, tag="r", name="r")
            nc.vector.stream_shuffle(r, psm, SWAP_MASK)
            t1 = rpool.tile([128, 512], F32, tag="t1", name="t1")
            nc.vector.tensor_tensor(out=t1, in0=psm, in1=cosT,
                                    op=mybir.AluOpType.mult)
            t2 = rpool.tile([128, 512], F32, tag="t2", name="t2")
            nc.vector.tensor_tensor(out=t2, in0=r, in1=sinT,
                                    op=mybir.AluOpType.mult)
            nc.vector.tensor_tensor(out=dest_slice, in0=t1, in1=t2,
                                    op=mybir.AluOpType.add)

        for _rep in range(reps):
            with ExitStack() as ph:
                xpool = ph.enter_context(tc.tile_pool(name="xp", bufs=1))
                xT_sb = [xpool.tile([128, KL], DT_MM, tag=f"xT{i}",
                                    name=f"xT{i}")
                         for i in range(8)]

                # ---- V first (attention needs all v-time tiles) ----
                for tt in range(8):
                    nc.sync.dma_start(
                        out=v1[tt].rearrange("p (h x) -> p h x",
                                             x=HD + 1)[:, :, HD:HD + 1],
                        in_=vone_d[:, tt * H:(tt + 1) * H])
                with tc.tile_pool(name="wv", bufs=1) as wvpool, \
                        tc.tile_pool(name="psV", bufs=1, space="PSUM") as psV:
                    for oh in range(2):
                        wv = [wvpool.tile([128, 512], DT_MM, tag=f"wv{i}",
                                          name=f"wv{i}")
                              for i in range(8)]
                        for ct in range(8):
                            r0 = ct * 128
                            c0 = 2 * C + oh * 512
                            if oh == 0:
                                nc.sync.dma_start(
                                    out=xT_sb[ct],
                                    in_=xT_d[r0:r0 + 128, :])
                            nc.sync.dma_start(
                                out=wv[ct], in_=wT_d[r0:r0 + 128, c0:c0 + 512])
                        pv = [psV.tile([128, 512], F32, tag=f"pv{i}",
                                       name=f"pv{i}")
                              for i in range(8)]
                        for ct in range(8):
                            for tt in range(8):
                                lhs = xT_sb[ct][:, tt * 128:(tt + 1) * 128]
                                nc.tensor.matmul(
                                    pv[tt], lhs, wv[ct],
                                    start=(ct == 0), stop=(ct == 7))
                        for tt in range(8):
                            dst = v1[tt].rearrange(
                                "p (h x) -> p h x", x=HD + 1)[
                                :, oh * 8:(oh + 1) * 8, 0:HD]
                            src = pv[tt].rearrange("p (h x) -> p h x", x=HD)
                            # NB: must stay on ACT -- DVE tensor_copy into
                            # this strided f32r tile miscomputes on HW
                            # (sim-exact, HW relerr 0.15). HW-validated path.
                            nc.scalar.copy(dst, src)

                # ---- K/Q per o-tile + attention for that head pair ----
                with ExitStack() as ph2:
                    wqk = ph2.enter_context(tc.tile_pool(name="wqk", bufs=2))
                    rpool = ph2.enter_context(tc.tile_pool(name="rp", bufs=2))
                    apool = ph2.enter_context(tc.tile_pool(name="att", bufs=1))
                    psm_p = ph2.enter_context(
                        tc.tile_pool(name="psm", bufs=2, space="PSUM"))
                    aps = ph2.enter_context(
                        tc.tile_pool(name="attps", bufs=1, space="PSUM"))
                    ypool = ph2.enter_context(
                        tc.tile_pool(name="yps", bufs=1, space="PSUM"))
                    for ot in range(8):
                        wkq = [wqk.tile([128, 256], DT_MM, tag=f"wkq{i}",
                                        name=f"wkq{i}")
                               for i in range(8)]
                        for ct in range(8):
                            r0 = ct * 128
                            nc.sync.dma_start(
                                out=wkq[ct],
                                in_=wT_d[r0:r0 + 128,
                                         ot * 256:(ot + 1) * 256])
                        wk = [w[:, 0:128] for w in wkq]
                        wq = [w[:, 128:256] for w in wkq]
                        # K o-tile in two 512-wide time halves
                        for th in range(2):
                            psm = psm_p.tile([128, 512], F32, tag="psm",
                                             name="psm")
                            for ct in range(8):
                                nc.tensor.matmul(
                                    psm, wk[ct],
                                    xT_sb[ct][:, th * 512:(th + 1) * 512],
                                    start=(ct == 0), stop=(ct == 7))
                            rope_evict(kT[ot][:, th * 512:(th + 1) * 512],
                                       psm, rpool,
                                       cosk_sb[:, th * 512:(th + 1) * 512],
                                       sink_sb[:, th * 512:(th + 1) * 512])
                        # Q o-tile (queries = x cols 512:1024)
                        psm = psm_p.tile([128, 512], F32, tag="psm",
                                         name="psm")
                        for ct in range(8):
                            nc.tensor.matmul(
                                psm, wq[ct], xT_sb[ct][:, 512:1024],
                                start=(ct == 0), stop=(ct == 7))
                        rope_evict(qT[ot], psm, rpool, cosq_sb, sinq_sb)

                        # attention for heads 2*ot (PE rows 0-63) and
                        # 2*ot+1 (rows 64-127): paired score matmuls land
                        # in disjoint row-groups and execute concurrently
                        for qbi in range(2):
                            qb = qbi * QB
                            # scores in three 2-subtile chunks per head
                            # (each chunk = 1 PSUM bank, double-buffered)
                            pes = [apool.tile([128, NSUB * QB], DT_MM,
                                              tag=f"pe{hh}", name=f"pe{hh}")
                                   for hh in range(2)]
                            yps = [ypool.tile([65, QB], F32, tag=f"yp{hh}",
                                              name=f"yp{hh}")
                                   for hh in range(2)]
                            for ck in range(3):
                                ps_pair = [aps.tile([128, 2 * QB], F32,
                                                    tag=f"ps{hh}",
                                                    name=f"ps{hh}")
                                           for hh in range(2)]
                                for si in range(2):
                                    s = ck * 2 + si
                                    k0 = qb + s * 128
                                    for hh in range(2):
                                        po = hh * 64
                                        nc.tensor.matmul(
                                            ps_pair[hh][:, si * QB:
                                                        (si + 1) * QB],
                                            kT[ot][po:po + 64, k0:k0 + 128],
                                            qT[ot][po:po + 64, qb:qb + QB],
                                            start=True, stop=True)
                                for hh in range(2):
                                    nc.scalar.activation(
                                        pes[hh][:, ck * 2 * QB:
                                                (ck + 1) * 2 * QB],
                                        ps_pair[hh],
                                        mybir.ActivationFunctionType.Exp)
                            mb0 = qbi * NSUB * QB
                            for hh in range(2):
                                pe = pes[hh]
                                nc.vector.tensor_tensor(
                                    out=pe[:, 0:2 * QB], in0=pe[:, 0:2 * QB],
                                    in1=mask_sb[:, mb0:mb0 + 2 * QB],
                                    op=mybir.AluOpType.mult)
                                nc.vector.tensor_tensor(
                                    out=pe[:, 4 * QB:6 * QB],
                                    in0=pe[:, 4 * QB:6 * QB],
                                    in1=mask_sb[:, mb0 + 4 * QB:mb0 + 6 * QB],
                                    op=mybir.AluOpType.mult)
                            for s in range(NSUB):
                                for hh in range(2):
                                    h = 2 * ot + hh
                                    vt = v1[qbi * 2 + s][
                                        :, (HD + 1) * h:(HD + 1) * h + HD + 1]
                                    nc.tensor.matmul(
                                        yps[hh], vt,
                                        pes[hh][:, s * QB:(s + 1) * QB],
                                        start=(s == 0), stop=(s == NSUB - 1))
                            for hh in range(2):
                                po = hh * 64
                                rd = apool.tile([1, QB], F32, tag="rd",
                                                name="rd")
                                nc.vector.reciprocal(rd, yps[hh][64:65, :])
                                rb = apool.tile([64, QB], F32, tag="rb",
                                                name="rb")
                                nc.gpsimd.partition_broadcast(rb, rd)
                                nc.vector.tensor_tensor(
                                    out=yT[ot][po:po + 64, qb:qb + QB],
                                    in0=yps[hh][0:64, :], in1=rb,
                                    op=mybir.AluOpType.mult)

            # ---------------- output projection ----------------------
            with ExitStack() as phd:
                wopool = phd.enter_context(tc.tile_pool(name="wo", bufs=1))
                opool = phd.enter_context(tc.tile_pool(name="ob", bufs=2))
                psO = phd.enter_context(
                    tc.tile_pool(name="psO", bufs=2, space="PSUM"))
                wp = [wopool.tile([128, C], DT_MM, tag=f"wo{i}",
                                  name=f"wo{i}")
                      for i in range(8)]
                for ct in range(8):
                    nc.sync.dma_start(
                        out=wp[ct], in_=wpT_d[ct * 128:(ct + 1) * 128, :])
                for tt in range(4):
                    po_ = psO.tile([128, C], F32, tag="psO", name="psO")
                    for ct in range(8):
                        lhs = yT[ct][:, tt * 128:(tt + 1) * 128]
                        st, sp = (ct == 0), (ct == 7)
                        for hh in range(2):
                            sl = slice(hh * 512, (hh + 1) * 512)
                            nc.tensor.matmul(
                                po_[:, sl], lhs, wp[ct][:, sl],
                                start=st, stop=sp)
                    ob = opool.tile([128, C], F32, tag="ob", name="ob")
                    nc.vector.tensor_tensor(
                        out=ob, in0=po_, in1=bias_sb,
                        op=mybir.AluOpType.add)
                    nc.sync.dma_start(
                        out=out_d[tt * 128:(tt + 1) * 128, :], in_=ob)

    nc.compile()
    return nc


# ---------------------------------------------------------------------
# host side
# ---------------------------------------------------------------------

def _trig_tables(positions, n):
    """cos / signed-sin tables in transposed layout [128, n].

    Row p corresponds to head-dim d = p % 64; freq index d//2.  The sin
    table carries the rotate_half sign: -sin on even d, +sin on odd d,
    so that q' = q*cos + swap_pairs(q)*sin_signed.
    """
    inv = 1.0 / (ROPE_BASE ** (np.arange(HD // 2, dtype=np.float32)
                               / (HD // 2)))
    freqs = positions.astype(np.float32)[None, :] * inv[:, None]  # [32, n]
    cos = np.repeat(np.cos(freqs), 2, axis=0)  # [64, n]
    sin = np.repeat(np.sin(freqs), 2, axis=0)
    sign = np.where(np.arange(64) % 2 == 0, -1.0, 1.0).astype(np.float32)
    sin = sin * sign[:, None]
    return (np.ascontiguousarray(np.tile(cos, (2, 1))),
            np.ascontiguousarray(np.tile(sin, (2, 1))))


def _masks(qs):
    """0/1 mask tiles [128, 12*QB]: scores^T layout [key-part, query-free]."""
    p = np.arange(128)[:, None]
    xx = np.arange(QB)[None, :]
    m = np.zeros((128, 12, QB), np.float32)
    col = 0
    for qb in (0, QB):
        for s in range(NSUB):
            pk = (qs - LOCAL_WINDOW) + qb + 128 * s + p  # global key pos
            band = (xx >= 128 * s + p - LOCAL_WINDOW) & (xx <= 128 * s + p)
            m[:, col, :] = (band & (pk >= 0)).astype(np.float32)
            col += 1
    return np.ascontiguousarray(m.reshape(128, 12 * QB))


def _host_inputs(x, Wqkv, Wproj, bproj):
    # 1/sqrt(hd) folded into the (linear) q projection
    Wq, Wk, Wv = Wqkv[0:C] * SCALE, Wqkv[C:2 * C], Wqkv[2 * C:3 * C]
    # slab layout: per o-tile [K(128) | Q(128)] interleaved, then V
    WqT, WkT, WvT = Wq.T, Wk.T, Wv.T
    slab = np.empty((C, 2 * C), np.float32)
    for ot in range(8):
        slab[:, ot * 256:ot * 256 + 128] = WkT[:, ot * 128:(ot + 1) * 128]
        slab[:, ot * 256 + 128:ot * 256 + 256] = \
            WqT[:, ot * 128:(ot + 1) * 128]
    wT = np.ascontiguousarray(
        np.concatenate([slab, WvT], axis=1)).astype(np.float32)
    wpT = np.ascontiguousarray(Wproj.T).astype(np.float32)
    biasb = np.ascontiguousarray(
        np.broadcast_to(bproj, (128, C))).astype(np.float32)

    in_maps = []
    for core in range(NCORES):
        b, qi = divmod(core, 4)
        qs = qi * QL
        qe = qs + QL
        lo = qe - KL
        xw = np.zeros((KL, C), np.float32)
        src_lo = max(lo, 0)
        xw[src_lo - lo:, :] = x[b, src_lo:qe, :]
        xT = np.ascontiguousarray(xw.T)

        cosq, sinq = _trig_tables(qs + np.arange(QL), QL)
        cosk, sink = _trig_tables(lo + np.arange(KL), KL)
        # validity column for [V|1]: 1.0 where the key row is a real
        # (non-padding) position, per v-time-tile, repeated per head
        jpos = lo + np.arange(KL)
        v8 = (jpos >= 0).astype(np.float32).reshape(8, 128)  # [tt, p]
        vone = np.ascontiguousarray(
            np.repeat(v8[:, :, None], H, axis=2)             # [tt, p, h]
            .transpose(1, 0, 2).reshape(128, 8 * H))         # [p, tt*h]
        in_maps.append({
            "xT": xT, "wT": wT, "wpT": wpT, "biasb": biasb,
            "cosq": cosq, "sinq": sinq, "cosk": cosk, "sink": sink,
            "mask": _masks(qs), "vone": vone,
        })
    return in_maps


_NC_CACHE = {}


def _get_nc(reps=1):
    key = (reps, DT_MM)
    if key not in _NC_CACHE:
        _NC_CACHE[key] = build_nc(reps=reps)
    return _NC_CACHE[key]


def kernel(x, Wqkv, Wproj, bproj):
    x = np.asarray(x, dtype=np.float32)
    Wqkv = np.asarray(Wqkv, dtype=np.float32)
    Wproj = np.asarray(Wproj, dtype=np.float32)
    bproj = np.asarray(bproj, dtype=np.float32)
    nc = _get_nc()
    in_maps = _host_inputs(x, Wqkv, Wproj, bproj)
    res = run_bass_kernel_spmd(nc, in_maps, list(range(NCORES)))
    out = np.empty((B, T, C), np.float32)
    for core in range(NCORES):
        b, qi = divmod(core, 4)
        out[b, qi * QL:(qi + 1) * QL, :] = res.results[core]["out"]
    return out

